# revision 1
# baseline (speedup 1.0000x reference)
"""Trainium2 Bass kernel for nn_Attention_326417514823.

Per-batch computation (B=8, N=2048, D=256), one batch per NeuronCore:
    S = Q @ K.T / sqrt(D)                  (N x N)
    S[q, :] = -1e9 where mask[q] == 0      (row masking by query index)
    A = softmax(S, axis=0)                 (normalize over q, per column k)
    A[q, :] = 0 where mask[q] == 0
    O = A @ V                              (N x D)

Algebra used on device: the softmax normalizer c[k] = sum_q E[q,k] is
per-column, so it folds into V (W[k,:] = V[k,:] / c[k]) and O = E @ W with
E = exp(S/16) * mask[q].  No max-subtraction is needed: scores/16 stay in
[-7, 7], and the reference's masked entries are exp(-1e9 - max) == 0
exactly in fp32, which the mask-multiply reproduces exactly (zero).

Device layout (transposed so the softmax reduction runs along the free axis
and neither matmul needs an on-chip transpose):
    ST[k, q] = KT.T @ QT   (KT = K.T, QT = Q.T, d on partitions)
    E[k, q]  = exp(ST/16) * mask_bcast              (bf16)
    c[k]     = sum_q E[k, q]  (fused accum in the DVE mask multiply)
    W[k, :]  = V[k, :] * (1/c[k])                   (bf16)
    OT[d, q] = sum_k W[k,d] * E[k,q]  (PSUM accumulation over k-blocks)
Host transposes OT back to O.

Pipelining: PSUM = 8 banks. 4 banks hold the q<1024 half of OT's
accumulators for the WHOLE kernel, so half of matmul-2 interleaves into
phase 1 (lagging LAG k-blocks behind the softmax pipeline). The score
tiles double-buffer in the other 4 banks; once phase 1 ends those 4 banks
are reused for the q>=1024 accumulators, accumulated chain-per-bank so
each store overlaps the remaining chains. DMA emissions are ordered by
first consumption (HWDGE ring prep ~625 ns each is a shared serial
resource, and transfers serialize at ~360 GB/s).
"""

import numpy as np
import ml_dtypes

B, N, D = 8, 2048, 256
NCORES = 8
P = 128          # partitions
MMN = 512        # matmul moving free dim (one PSUM bank of fp32)
KB = N // P      # 16 k-blocks
NCH = N // MMN   # 4 512-chunks along q
DT = D // P      # 2 d-tiles
LAG = 4          # k-blocks of slack before interleaved matmul-2 consumes W
STT_SPLIT = False  # split mask-multiply per half: measured slower (DVE op overhead)

# "f32r": fp32 storage everywhere, TF32-class matmuls (1 cycle/row at
#         N>=256 per the TRN2 cost model) — most accurate (~3e-4).
# "mixed": Q/K in bf16 (halves the startup DMA-bus time; scores lose ~2e-3)
#         but E/W/c stay fp32r so the softmax/output path stays fp32-clean.
# "bf16": everything bf16 (~5e-3).
DTYPE_MODE = "f32r"

_cached = None


def _build():
    import concourse.bacc as bacc
    import concourse.mybir as mybir
    import concourse.tile as tile

    f32 = mybir.dt.float32
    bf16 = mybir.dt.bfloat16
    mmdt = bf16 if DTYPE_MODE == "bf16" else mybir.dt.float32r
    qkdt = mybir.dt.float32r if DTYPE_MODE == "f32r" else bf16
    MULT = mybir.AluOpType.mult
    EXP = mybir.ActivationFunctionType.Exp

    nc = bacc.Bacc()
    kt = nc.dram_tensor("kt", [D, N], qkdt, kind="ExternalInput")
    qt = nc.dram_tensor("qt", [D, N], qkdt, kind="ExternalInput")
    v = nc.dram_tensor("v", [N, D], f32, kind="ExternalInput")
    mb = nc.dram_tensor("mb", [1, N], bf16, kind="ExternalInput")
    ot = nc.dram_tensor("ot", [D, N], f32, kind="ExternalOutput")

    with tile.TileContext(nc) as tc:
        with (
            tc.tile_pool(name="const", bufs=1) as constp,
            tc.tile_pool(name="epool", bufs=1) as epool,
            tc.tile_pool(name="wpool", bufs=1) as wpool,
            tc.tile_pool(name="vpool", bufs=3) as vpool,
            tc.tile_pool(name="cpool", bufs=3) as cpool,
            tc.tile_pool(name="outp", bufs=6) as outp,
            # q<1024 OT accumulators live for the whole kernel (banks 0-3)
            tc.tile_pool(name="psA", bufs=1, space="PSUM") as psA,
        ):
            # inputs, chunked so the first matmuls start after ~128KB of DMA
            kt_ch = [[constp.tile([P, MMN], qkdt, name=f"ktc{d}_{j}")
                      for j in range(NCH)] for d in range(DT)]
            qt_ch = [[constp.tile([P, MMN], qkdt, name=f"qtc{d}_{j}")
                      for j in range(NCH)] for d in range(DT)]
            # DMA-ring choreography: kb=0 needs kt[*][0] and ALL qt chunks
            # immediately; kt[*][j] only at kb=4j; v at the k-block pace.
            # kt j0 goes on ScalarE's DGE ring (idle until the first exp) in
            # parallel with qt j0 on the SP ring; later kt chunks are emitted
            # inside the loop so v/mask don't queue behind them.
            def load_kt(d, j):
                nc.sync.dma_start(
                    kt_ch[d][j][:], kt[d * P:(d + 1) * P, j * MMN:(j + 1) * MMN])

            for d in range(DT):
                nc.scalar.dma_start(
                    kt_ch[d][0][:], kt[d * P:(d + 1) * P, 0:MMN])
            # exact consumption order of kb=0's matmuls: ch0 uses
            # (j0,d0),(j1,d0),(j0,d1),(j1,d1); ch1 uses (j2,d0),(j3,d0),...
            for j, d in [(0, 0), (1, 0), (0, 1), (1, 1),
                         (2, 0), (3, 0), (2, 1), (3, 1)]:
                nc.sync.dma_start(
                    qt_ch[d][j][:], qt[d * P:(d + 1) * P, j * MMN:(j + 1) * MMN])
            mbc = constp.tile([P, N], bf16, name="mbc")
            nc.sync.dma_start(mbc[:], mb[0:1, :].partition_broadcast(P))

            accA = [[psA.tile([P, MMN], f32, name=f"accA{dh}_{qc}")
                     for qc in range(2)] for dh in range(DT)]

            # Warm the PE (p-state / HAM ramp) during the initial DMA wait:
            # dummy matmuls on a zeroed tile into accA[0][0], whose garbage
            # is cleared by the first real start=True accumulation.
            zs = constp.tile([P, P], f32, name="zs")
            nc.vector.memset(zs[:], 0.0)
            zsr = zs[:].bitcast(mmdt) if mmdt != bf16 else zs[:, 0:P // 2].bitcast(bf16)
            for _ in range(16):
                nc.tensor.matmul(accA[0][0][:, 0:zsr.shape[1]], zsr, zsr,
                                 start=True, stop=True)

            e_all = [None] * KB
            w_all = [None] * KB

            def mm2(acc, kb, dh, qci):
                nc.tensor.matmul(
                    acc[:],
                    w_all[kb][:, dh * P:(dh + 1) * P],
                    e_all[kb][:, qci * MMN:(qci + 1) * MMN],
                    start=(kb == 0),
                    stop=(kb == KB - 1),
                )

            # V loads batched 4 k-blocks per DMA: one [128, 4*D] tile per
            # group, free dim laid out as (sub, d)
            v_grps = {}

            def load_vg(g):
                if g < KB // 4 and g not in v_grps:
                    v_g = vpool.tile([P, 4, D], f32, name="v_g")
                    src = v[g * 4 * P:(g + 1) * 4 * P, :].rearrange(
                        "(s p) d -> p s d", p=P)
                    nc.sync.dma_start(v_g[:], src)
                    v_grps[g] = v_g

            def v_slice(kb):
                return v_grps[kb // 4][:, kb % 4, :]

            load_vg(0)

            def mm1_exp_half(kb, ch, psS, e_kb):
                # one q-half = two 512-wide score buffers (1 PSUM bank each)
                for ch4 in (ch * 2, ch * 2 + 1):
                    st = psS.tile([P, MMN], f32, name="st")
                    for d in range(DT):
                        nc.tensor.matmul(
                            st[:],
                            kt_ch[d][kb // 4][:, (kb % 4) * P:(kb % 4 + 1) * P],
                            qt_ch[d][ch4][:],
                            start=(d == 0),
                            stop=(d == DT - 1),
                        )
                    nc.scalar.activation(
                        e_kb[:, ch4 * MMN:(ch4 + 1) * MMN], st[:],
                        EXP, scale=1.0 / 16.0)

            with tc.tile_pool(name="psS", bufs=4, space="PSUM") as psS:
                e_warm = [epool.tile([P, N], mmdt, name=f"e{kb}")
                          for kb in range(4)]
                for kb, ch in [(0, 0), (1, 0), (2, 0), (0, 1), (1, 1), (2, 1),
                               (3, 0), (3, 1)]:
                    # the q>=1024 input chunks are still in flight on the DMA
                    # bus while kb 0-2's q<1024 halves run
                    mm1_exp_half(kb, ch, psS, e_warm[kb])

                for kb in range(KB):
                    if kb % 4 == 1:
                        load_vg(kb // 4 + 1)
                    if kb in (0, 4, 8):
                        for d in range(DT):
                            load_kt(d, kb // 4 + 1)
                    if kb < 4:
                        e_kb = e_warm[kb]
                    else:
                        e_kb = epool.tile([P, N], mmdt, name=f"e{kb}")
                        for ch in range(2):
                            mm1_exp_half(kb, ch, psS, e_kb)
                    if STT_SPLIT:
                        H = N // 2
                        c_kb = cpool.tile([P, 1], f32, name="c")
                        c_lo = cpool.tile([P, 1], f32, name="c_lo")
                        nc.vector.scalar_tensor_tensor(
                            e_kb[:, 0:H], e_kb[:, 0:H], 1.0, mbc[:, 0:H],
                            MULT, MULT, accum_out=c_lo[:])
                        c_hi = cpool.tile([P, 1], f32, name="c_hi")
                        nc.vector.scalar_tensor_tensor(
                            e_kb[:, H:N], e_kb[:, H:N], 1.0, mbc[:, H:N],
                            MULT, MULT, accum_out=c_hi[:])
                        nc.vector.tensor_tensor(
                            c_kb[:], c_lo[:], c_hi[:], mybir.AluOpType.add)
                    else:
                        c_kb = cpool.tile([P, 1], f32, name="c")
                        nc.vector.scalar_tensor_tensor(
                            e_kb[:], e_kb[:], 1.0, mbc[:], MULT, MULT,
                            accum_out=c_kb[:])
                    rc = cpool.tile([P, 1], f32, name="rc")
                    nc.vector.reciprocal(rc[:], c_kb[:])
                    w_kb = wpool.tile([P, D], mmdt, name=f"w{kb}")
                    nc.vector.tensor_scalar_mul(w_kb[:], v_slice(kb), rc[:])
                    e_all[kb] = e_kb
                    w_all[kb] = w_kb

                    # interleaved half of matmul-2, LAG k-blocks behind
                    if kb >= LAG:
                        for dh in range(DT):
                            for qci in range(2):
                                mm2(accA[dh][qci], kb - LAG, dh, qci)
                for j in range(KB - LAG, KB):
                    for dh in range(DT):
                        for qci in range(2):
                            mm2(accA[dh][qci], j, dh, qci)

            # q<1024 results: copy + store (overlaps the q>=1024 matmuls)
            def store(acc, dh, qci, engine):
                o_sb = outp.tile([P, MMN], f32, name="o_sb")
                if engine == "act":
                    nc.scalar.copy(o_sb[:], acc[:])
                else:
                    nc.vector.tensor_copy(o_sb[:], acc[:])
                nc.sync.dma_start(
                    ot[dh * P:(dh + 1) * P, qci * MMN:(qci + 1) * MMN], o_sb[:])

            with tc.tile_pool(name="psB", bufs=4, space="PSUM") as psB:
                def accb_tile():
                    return psB.tile([P, MMN], f32, name="accB", tag="accB")
                for dh in range(DT):
                    for qci in range(2):
                        store(accA[dh][qci], dh, qci, "act" if dh == 0 else "dve")
                # chain-per-accumulator so each finishes early and its copy
                # overlaps the remaining accumulation chains
                for qci in range(2, NCH):
                    for dh in range(DT):
                        if (qci, dh) != (NCH - 1, DT - 1):
                            acc = accb_tile()
                            for kb in range(KB):
                                mm2(acc, kb, dh, qci)
                            store(acc, dh, qci, "act" if dh == 0 else "dve")
                        else:
                            # very last output: two half-width chains in
                            # SEPARATE banks (the second reuses the first
                            # finished chain's bank), so half A's copy+DMA
                            # fixed costs (~2.9us) hide under half B's MMs
                            o_sb = outp.tile([P, MMN], f32, name="o_sb")
                            # halves no narrower than 256: f32r matmuls drop
                            # to 1/4 rate below a 256-wide moving dim
                            for lo, W_ in ((0, 256), (256, 256)):
                                acc = accb_tile()
                                for kb in range(KB):
                                    nc.tensor.matmul(
                                        acc[:, 0:W_],
                                        w_all[kb][:, dh * P:(dh + 1) * P],
                                        e_all[kb][:, qci * MMN + lo:
                                                  qci * MMN + lo + W_],
                                        start=(kb == 0),
                                        stop=(kb == KB - 1),
                                    )
                                nc.vector.tensor_copy(o_sb[:, lo:lo + W_],
                                                      acc[:, 0:W_])
                                nc.sync.dma_start(
                                    ot[dh * P:(dh + 1) * P,
                                       qci * MMN + lo:qci * MMN + lo + W_],
                                    o_sb[:, lo:lo + W_])

    nc.compile()
    return nc


def _get_nc():
    global _cached
    if _cached is None:
        _cached = _build()
    return _cached


def kernel(key, query, value, mask):
    from concourse.bass_utils import run_bass_kernel_spmd

    nc = _get_nc()
    bf = ml_dtypes.bfloat16
    key = np.asarray(key, dtype=np.float32)
    query = np.asarray(query, dtype=np.float32)
    value = np.asarray(value, dtype=np.float32)
    mask = np.asarray(mask)

    iodt = np.float32 if DTYPE_MODE == "f32r" else bf
    in_maps = []
    for b in range(B):
        in_maps.append({
            "kt": np.ascontiguousarray(key[b].T).astype(iodt),
            "qt": np.ascontiguousarray(query[b].T).astype(iodt),
            "v": np.ascontiguousarray(value[b]),
            "mb": np.ascontiguousarray(mask[b]).astype(bf),
        })
    res = None
    for attempt in range(4):
        try:
            res = run_bass_kernel_spmd(nc, in_maps, core_ids=list(range(NCORES)))
            break
        except Exception:
            # Transient "accelerator device unrecoverable" states wedge the
            # PJRT client but not the device: tear down the backend and retry.
            if attempt == 3:
                raise
            import time
            time.sleep(10 * (attempt + 1))
            try:
                import jax.extend.backend as _jb
                _jb.clear_backends()
                import jax
                jax.clear_caches()
            except Exception:
                pass
    out = np.empty((B, N, D), np.float32)
    for b in range(B):
        out[b] = res.results[b]["ot"].T
    return out



# revision 48
# speedup vs baseline: 1.9241x; 1.9241x over previous
"""Trainium2 Bass kernel for nn_Attention_326417514823.

Per-batch computation (B=8, N=2048, D=256), one batch per NeuronCore:
    S = Q @ K.T / sqrt(D);  S[q,:] = -inf where mask[q]==0
    A = softmax(S, axis=0)  (normalize over q, per key column k)
    A[q,:] = 0 where mask[q]==0;  O = A @ V

Host-side restructuring (all exact):
  * q-compaction: masked q rows produce zero output rows and are excluded
    from the softmax normalizer, so the kernel only processes the nU
    unmasked q columns, padded to NQ (multiple of 64).  Pad columns carry
    Q=0 => exp(0)=1, removed from the normalizer via the host-computed
    count correction nmv = NQ - nU.  Host scatters rows back at the end.
  * fp8 residual split: X =~ X8 + dX8 (both e4m3) gives near-bf16-accurate
    matmuls out of fp8 DoubleRow instructions, which the PE runs at
    0.5 cycles/row (4x cheaper than bf16 per unit of contraction).

Device layout (k on partitions, q on the free axis; d-halves in dim1 for
DoubleRow):
    ST[k,q] = K8.Q8 + K8.dQ8 + dK8.Q8            (3 DR chains, PSUM f32)
    E16[k,q] = bf16(exp(ST/16))                  (Act engine, pure exp)
    c~[k] = sum_q E16 (bf16 DVE reduce at 4x);  rc = 1/(c~ - nmv)
    W16 = bf16(V16 * rc)
    OT[d,q] = sum_k W16^T E16                    (bf16 matmul chains)

Schedule: k-block-inner loop; scores double-buffered in 2x3 PSUM banks;
two [128,RES_Q] chain accumulators stay resident and accumulate
incrementally with a small k-block lag; the q>=RES_Q columns replay from
the persistent E16 tiles at the tail.  (GPSIMD compute is rejected by the
walrus engine checks, so everything elementwise lives on Act/DVE.)
"""

import numpy as np
import ml_dtypes

B, N, D = 8, 2048, 256
NCORES = 8
P = 128
KB = N // P          # 16 k-blocks
NPAIR = KB // 2      # 8 k-pairs for DoubleRow
MMW = 256            # max moving width per DR matmul (rhs free = 2*MMW)
RES_Q = 512          # resident chain width (1 PSUM bank of f32)
CHAIN_LAG = 3        # k-blocks of slack before resident chains consume W16

_cached = {}


def _q_chunks(nq, width):
    out = []
    q0 = 0
    while q0 < nq:
        out.append((q0, min(width, nq - q0)))
        q0 += width
    return out


def _build(NQ):
    import concourse.bacc as bacc
    import concourse.mybir as mybir
    import concourse.tile as tile

    f32 = mybir.dt.float32
    bf16 = mybir.dt.bfloat16
    fp8 = mybir.dt.float8e4
    MULT = mybir.AluOpType.mult
    SUB = mybir.AluOpType.subtract
    EXP = mybir.ActivationFunctionType.Exp
    DR = mybir.MatmulPerfMode.DoubleRow

    DQ = NQ - RES_Q            # deferred q width
    SCW = ((NQ * 4 + 6143) // 6144) * 1536  # score cols, 3-bank multiple

    nc = bacc.Bacc()
    kt8 = nc.dram_tensor("kt8", [P, 2, N], fp8, kind="ExternalInput")
    dkt8 = nc.dram_tensor("dkt8", [P, 2, N], fp8, kind="ExternalInput")
    qt8 = nc.dram_tensor("qt8", [P, 2, NQ], fp8, kind="ExternalInput")
    dqt8 = nc.dram_tensor("dqt8", [P, 2, NQ], fp8, kind="ExternalInput")
    vt = nc.dram_tensor("vt", [P, KB * D], bf16, kind="ExternalInput")
    nmv = nc.dram_tensor("nmv", [1, 1], f32, kind="ExternalInput")
    ot = nc.dram_tensor("ot", [2 * P, NQ], bf16, kind="ExternalOutput")

    with tile.TileContext(nc) as tc:
        with (
            tc.tile_pool(name="const", bufs=1) as constp,
            tc.tile_pool(name="e16p", bufs=1) as e16p,
            tc.tile_pool(name="redp", bufs=2) as redp,
            tc.tile_pool(name="wp", bufs=1) as wp,
            tc.tile_pool(name="outp", bufs=4) as outp,
            tc.tile_pool(name="psS", bufs=2, space="PSUM") as psS,
            tc.tile_pool(name="psC", bufs=1, space="PSUM") as psC,
        ):
            # ---- input staging -------------------------------------------
            # HWDGE prep is ~628ns/DMA and serializes globally, so inputs
            # travel in 8 DMAs ordered by first consumption: the kb0-3 K
            # chunks and the q operands first, everything else behind.
            kt_sb = constp.tile([P, 2, N], fp8, name="kt_sb")
            dkt_sb = constp.tile([P, 2, N], fp8, name="dkt_sb")
            qt_sb = constp.tile([P, 2, NQ], fp8, name="qt_sb")
            dqt_sb = constp.tile([P, 2, NQ], fp8, name="dqt_sb")
            v_sb = constp.tile([P, KB * D], bf16, name="v_sb")
            nmvb = constp.tile([P, 1], f32, name="nmvb")

            s03 = slice(0, 4 * P)
            s47 = slice(4 * P, 8 * P)
            s8f = slice(8 * P, N)
            # all input DMAs ride the SP ring: a dma_start holds the
            # issuing engine's sequencer until its HWDGE prep completes
            # (625ns each, globally serialized), and SP has nothing else
            nc.sync.dma_start(kt_sb[:, :, s03], kt8[:, :, s03])
            nc.sync.dma_start(qt_sb[:], qt8[:, :, :])
            nc.sync.dma_start(dqt_sb[:], dqt8[:, :, :])
            nc.sync.dma_start(dkt_sb[:, :, s03], dkt8[:, :, s03])
            nc.sync.dma_start(v_sb[:, 0:4 * D], vt[:, 0:4 * D])
            nc.sync.dma_start(nmvb[:], nmv[0:1, :].partition_broadcast(P))
            nc.sync.dma_start(kt_sb[:, :, s47], kt8[:, :, s47])
            nc.sync.dma_start(dkt_sb[:, :, s47], dkt8[:, :, s47])
            nc.sync.dma_start(v_sb[:, 4 * D:8 * D], vt[:, 4 * D:8 * D])
            nc.sync.dma_start(kt_sb[:, :, s8f], kt8[:, :, s8f])
            nc.sync.dma_start(dkt_sb[:, :, s8f], dkt8[:, :, s8f])
            nc.sync.dma_start(v_sb[:, 8 * D:], vt[:, 8 * D:])

            c16 = constp.tile([P, KB], f32, name="c16")
            cm16 = constp.tile([P, KB], f32, name="cm16")
            rc16 = constp.tile([P, KB], f32, name="rc16")

            # resident OT chain accumulators (q < RES_Q), 1 bank each
            ct = [psC.tile([P, RES_Q], f32, name=f"ct{dh}") for dh in range(2)]

            # warm the PE p-state during the input DMA wait; garbage lands in
            # ct[0] and is cleared by the chain's first start=True matmul
            zs = constp.tile([P, 2, P], fp8, name="zs")
            nc.vector.memset(zs[:], 0.0)
            for _ in range(20):
                nc.tensor.matmul(ct[0][:, 0:P], zs[:], zs[:],
                                 start=True, stop=True, perf_mode=DR)

            w16 = [None] * KB
            e16_hist = [None] * KB

            def chain_mm(kbl):
                for dh in range(2):
                    mi = nc.tensor.matmul(
                        ct[dh][:, 0:RES_Q],
                        w16[kbl][:, dh * P:(dh + 1) * P],
                        e16_hist[kbl][:, 0:RES_Q],
                        start=(kbl == 0),
                        stop=False,
                    )
                    # slack-filler: prefer mm1 when both are ready (but
                    # late chains stay prompt so the close isn't delayed)
                    mi.ins.bass_priority = (mi.ins.bass_priority or 0) + 200

            for kb in range(KB):
                g, kt = kb // 2, kb % 2

                # mm1: 3-term DR into a 3-bank score tile (term order matches
                # the input DMA arrival order)
                sc = psS.tile([P, SCW], f32, name="sc")
                terms = ((kt_sb, qt_sb), (kt_sb, dqt_sb), (dkt_sb, qt_sb))
                # term-major: each input DMA unblocks a full sweep of
                # instructions (matters for the kb0/kb1 startup ramp);
                # within each PSUM chunk the accumulation order still runs
                # term 0 -> 1 -> 2
                for ti, (lt, rt) in enumerate(terms):
                    for q0, cw in _q_chunks(NQ, MMW):
                        nc.tensor.matmul(
                            sc[:, q0:q0 + cw],
                            lt[:, :, kb * P:(kb + 1) * P],
                            rt[:, :, q0:q0 + cw],
                            start=(ti == 0),
                            stop=(ti == 2),
                            perf_mode=DR,
                        )

                e16 = e16p.tile([P, NQ], bf16, name=f"e16_{kb}")
                e16_hist[kb] = e16
                nc.scalar.activation(e16[:], sc[:, 0:NQ], EXP, scale=1.0 / 16.0)

                # resident chains (lagged) run in the exp shadow on the PE
                if kb >= CHAIN_LAG:
                    chain_mm(kb - CHAIN_LAG)

                # c~[kb]: bf16 reduce on DVE (2-byte SBUF operands hit the
                # 4x DVE mode); the copy output is a throwaway
                red = redp.tile([P, NQ], bf16, name="red")
                nc.vector.tensor_scalar(
                    red[:], e16[:], 1.0, 0.0, MULT,
                    mybir.AluOpType.add, accum_out=c16[:, kb:kb + 1])

                # c correction + W tile for this k-block
                s1 = slice(kb, kb + 1)
                nc.vector.tensor_scalar(
                    cm16[:, s1], c16[:, s1], nmvb[:], None, SUB)
                nc.vector.reciprocal(rc16[:, s1], cm16[:, s1])
                vsl = v_sb[:, kb * D:(kb + 1) * D]
                w16[kb] = wp.tile([P, D], bf16, name=f"w16_{kb}")
                nc.vector.tensor_scalar(
                    w16[kb][:], vsl, rc16[:, s1], None, MULT)


            # ---- tail ----------------------------------------------------
            def store(acc, dh, q0, cw):
                o_sb = outp.tile([P, RES_Q], bf16, name="o_sb")
                nc.scalar.mul(o_sb[:, 0:cw], acc[:, 0:cw], 1.0)
                nc.sync.dma_start(
                    ot[dh * P:(dh + 1) * P, q0:q0 + cw], o_sb[:, 0:cw])

            # deferred chunks replay from the persistent E16 tiles in banks
            # freed by the last two score buffers; a large prefix of each
            # chain is issued first (operands all ready, fills the PE while
            # the last exps/conversions drain), then the resident close +
            # stores, then the chain tails
            dq_chunks = _q_chunks(DQ, RES_Q)
            wide_cw = dq_chunks[0][1]
            dacc = [psS.tile([P, SCW], f32, name="sc") for _ in range(2)]

            def deferred_layout():
                # only the wide chunk lives in the score-pool banks; the
                # narrow remainder accumulates in the psC banks freed by the
                # resident stores, so the wide stores pipeline ahead of it
                for dh in range(2):
                    yield dacc[dh], 0, 0, wide_cw, dh

            SPLIT_KB = KB - 6
            for acc, aq0, q0, cw, dh in deferred_layout():
                for kb in range(SPLIT_KB):
                    nc.tensor.matmul(
                        acc[:, aq0:aq0 + cw],
                        w16[kb][:, dh * P:(dh + 1) * P],
                        e16_hist[kb][:, RES_Q + q0:RES_Q + q0 + cw],
                        start=(kb == 0),
                        stop=False,
                    )

            # resident chain tail closes next (it gates the resident stores)
            for kbl in range(KB - CHAIN_LAG, KB):
                for dh in range(2):
                    nc.tensor.matmul(
                        ct[dh][:, 0:RES_Q],
                        w16[kbl][:, dh * P:(dh + 1) * P],
                        e16_hist[kbl][:, 0:RES_Q],
                        start=False,
                        stop=(kbl == KB - 1),
                    )
            for dh in range(2):
                store(ct[dh], dh, 0, RES_Q)

            # per dh: one [P, DQ] staging tile collects the wide chunk and
            # the narrow remainder, then ONE DMA ships it (HWDGE preps are
            # the tail's serial resource)
            o_dsb = [outp.tile([P, DQ], bf16, name=f"o_dsb{dh}")
                     for dh in range(2)]
            for acc, aq0, q0, cw, dh in deferred_layout():
                for kb in range(SPLIT_KB, KB):
                    nc.tensor.matmul(
                        acc[:, aq0:aq0 + cw],
                        w16[kb][:, dh * P:(dh + 1) * P],
                        e16_hist[kb][:, RES_Q + q0:RES_Q + q0 + cw],
                        start=False,
                        stop=(kb == KB - 1),
                    )
                if dh == 0:
                    nc.scalar.mul(o_dsb[dh][:, 0:cw], acc[:, 0:cw], 1.0)
                else:
                    nc.vector.tensor_scalar(
                        o_dsb[dh][:, 0:cw], acc[:, 0:cw], 1.0, None, MULT)

            # narrow remainder: full chains in the freed resident banks
            if DQ > wide_cw:
                r0, rw = wide_cw, DQ - wide_cw
                for dh in range(2):
                    acc = psC.tile([P, RES_Q], f32, name=f"ct{dh}")
                    for kb in range(KB):
                        nc.tensor.matmul(
                            acc[:, 0:rw],
                            w16[kb][:, dh * P:(dh + 1) * P],
                            e16_hist[kb][:, RES_Q + r0:RES_Q + r0 + rw],
                            start=(kb == 0),
                            stop=(kb == KB - 1),
                        )
                    if dh == 0:
                        nc.scalar.mul(
                            o_dsb[dh][:, r0:r0 + rw], acc[:, 0:rw], 1.0)
                    else:
                        nc.vector.tensor_scalar(
                            o_dsb[dh][:, r0:r0 + rw], acc[:, 0:rw], 1.0,
                            None, MULT)
            for dh in range(2):
                nc.sync.dma_start(
                    ot[dh * P:(dh + 1) * P, RES_Q:NQ], o_dsb[dh][:])

    nc.compile()
    return nc


def _get_nc(NQ=None):
    if NQ is None:
        if not _cached:
            raise RuntimeError("kernel not built yet")
        return next(iter(_cached.values()))
    if NQ not in _cached:
        _cached[NQ] = _build(NQ)
    return _cached[NQ]


def kernel(key, query, value, mask):
    from concourse.bass_utils import run_bass_kernel_spmd

    fp8 = ml_dtypes.float8_e4m3fn
    bf = ml_dtypes.bfloat16
    key = np.asarray(key, dtype=np.float32)
    query = np.asarray(query, dtype=np.float32)
    value = np.asarray(value, dtype=np.float32)
    mask = np.asarray(mask)

    idxs = [np.nonzero(mask[b, 0].astype(bool))[0] for b in range(B)]
    nU_max = max(len(ix) for ix in idxs)
    NQ = max(RES_Q + 64, ((nU_max + 7) // 8) * 8)
    nc = _get_nc(NQ)

    def split8(x):
        x8 = x.astype(fp8)
        dx8 = (x - x8.astype(np.float32)).astype(fp8)
        return x8, dx8

    in_maps = []
    for b in range(B):
        ix = idxs[b]
        nU = len(ix)
        qc = np.zeros((NQ, D), np.float32)
        qc[:nU] = query[b][ix]
        # [P, 2, cols] layouts: dim1 = d half (for QK) with d on partitions
        qt = np.ascontiguousarray(qc.T).reshape(2, P, NQ).transpose(1, 0, 2)
        kt = np.ascontiguousarray(key[b].T).reshape(2, P, N).transpose(1, 0, 2)
        qt8, dqt8 = split8(np.ascontiguousarray(qt))
        kt8, dkt8 = split8(np.ascontiguousarray(kt))
        vt = value[b].reshape(KB, P, D).transpose(1, 0, 2).reshape(P, KB * D)
        in_maps.append({
            "kt8": kt8, "dkt8": dkt8, "qt8": qt8, "dqt8": dqt8,
            "vt": np.ascontiguousarray(vt).astype(bf),
            "nmv": np.full((1, 1), float(NQ - nU), np.float32),
        })

    res = None
    for attempt in range(4):
        try:
            res = run_bass_kernel_spmd(nc, in_maps, core_ids=list(range(NCORES)))
            break
        except Exception:
            if attempt == 3:
                raise
            import time
            time.sleep(10 * (attempt + 1))
            try:
                import jax.extend.backend as _jb
                _jb.clear_backends()
                import jax
                jax.clear_caches()
            except Exception:
                pass

    out = np.zeros((B, N, D), np.float32)
    for b in range(B):
        ix = idxs[b]
        otb = res.results[b]["ot"].astype(np.float32)  # [2P, NQ]
        o = np.concatenate([otb[0:P], otb[P:2 * P]], axis=0).T  # [NQ, D]
        out[b][ix] = o[:len(ix)]
    return out
